# revision 11
# baseline (speedup 1.0000x reference)
"""Trainium2 kernel for nn_DeepLinearTimeSeries.

The reference network is a 400-layer *linear* residual MLP: every step is
x <- x @ (W_i^T) [+ 0.1 * carry], with no nonlinearities anywhere. The whole
stack therefore collapses algebraically to a single matrix:

    out = x @ M_total,   M_total = T_enc @ T_temp @ T_dec @ W_out^T  (64 x 1)

We fold the 400 64x64 factors on the host (trivial FLOPs), then run the
remaining memory-bound pass y = x @ m on 8 NeuronCores, data-parallel over
the batch dim (sharding_hint).

Device kernel v3 (PE matvec, fp16 stream):
  * x is packed on the host to fp16 [128, 16384] per core: column n holds
    rows (2n, 2n+1) interleaved over h -- k = parity*64 + h. This halves the
    HBM stream (4 MiB/core) and puts the contraction dim on SBUF partitions
    so the TensorEngine does the multiply+reduce:
        y[2n+p] = sum_k lhsT[k, p] * xpack[k, n]
  * lhsT [128, 32] holds the (even, odd) m-pair in columns (0,1) AND a
    duplicate in columns (4,5). 32 matmuls of N=512 go to PSUM regions
    [32a : 32a+32, bank b] with column-group a = i%4, bank b = i//4, so
    every PSUM bank is written across all 128 partitions and the 4
    column-group tiles execute CONCURRENTLY in the PE array (~4ns stagger).
    Useful rows per group alternate between the (0,1) and (4,5) pair so the
    4 output slices {0,1},{36,37},{64,65},{100,101} map to 4 *distinct*
    SDMA engines (pairs p and p+32 share an engine).
  * PSUM evacuation: per chunk, DVE copies the left 512 columns and ACT the
    right 512, casting f32 -> fp16 (halves the SBUF write traffic that
    competes with the input DMA stream for the 435 GB/s fabric).
  * y leaves as fp16 in 2 waves of 4 DMAs (wave 0 after chunks 0-1 are
    evacuated, hidden under the stream; wave 1 is the only tail). Host
    un-permutes and upcasts.
  * 6 warmup matmuls on garbage data at kernel start open the PE HAM clock
    gate before the real matmuls; the column-group concurrency gives PE a
    4x margin over the stream even when cold.
"""

import contextlib

import numpy as np

import concourse.bass as bass
import concourse.mybir as mybir
from concourse.bass_utils import run_bass_kernel_spmd

# Problem constants (hardcoded per harness contract).
B, S, H = 128, 2048, 64
N_CORES = 8
RW = np.float32(0.1)
ROWS = B * S // N_CORES          # 32768 rows per core
P = 128                          # SBUF partitions = 2 parities x 64 h
NCOL = ROWS // 2                 # 16384 packed moving columns per core
NCHUNK = 4
CCOL = NCOL // NCHUNK            # 4096 columns per DMA chunk (1 MiB fp16)
MM_N = 512                       # moving free dim per matmul (1 PSUM bank)
MM_PER_CHUNK = CCOL // MM_N      # 8
N_MM = NCHUNK * MM_PER_CHUNK     # 32
M_PAD = 32                       # stationary cols: pairs at (0,1),(4,5) + 0s
N_WARM = 6                       # HAM warmup matmuls
YCOL = NCOL // NCHUNK            # 4096 y columns per output row-pair
# Output slice base partitions: column-group a uses pair (0,1) for even a
# and (4,5) for odd a, landing the 4 slices on 4 distinct SDMA engines.
Y_BASE = [0, 36, 64, 100]
FP16 = mybir.dt.float16
FP32 = mybir.dt.float32

# Extra kwargs for run_bass_kernel_spmd (test harness sets these for tracing).
RUN_KWARGS: dict = {}


def _collapse_weights(W_enc, W_temp, W_dec, W_out):
    """Fold the full linear stack into a single [H] f32 vector."""
    eye = np.eye(H, dtype=np.float32)

    def block_mat(Ws):
        # x1 = x0 W0^T ; x2 = x1 W1^T + 0.1 x0 ; then x <- x (Wi^T + 0.1 I)
        T = Ws[0].T @ Ws[1].T + RW * eye
        for Wi in Ws[2:]:
            T = T @ (Wi.T + RW * eye)
        return T

    M = block_mat(W_enc) @ block_mat(W_temp)
    for Wd in W_dec:
        M = M @ (Wd.T + RW * eye)
    return (M @ W_out.T).astype(np.float32).reshape(H)  # [H]


def _build_bass():
    nc = bass.Bass()
    xp = nc.dram_tensor("xp", [P, NCOL], FP16, kind="ExternalInput")
    w = nc.dram_tensor("w", [P, M_PAD], FP16, kind="ExternalInput")
    y = nc.dram_tensor("y", [8, YCOL], FP16, kind="ExternalOutput")

    with contextlib.ExitStack() as ctx:
        w_sb = ctx.enter_context(nc.sbuf_tensor("w_sb", [P, M_PAD], FP16))
        can_sb = ctx.enter_context(nc.sbuf_tensor("can_sb", [P, M_PAD], FP16))
        x_sb = ctx.enter_context(nc.sbuf_tensor("x_sb", [P, NCOL], FP16))
        y_sb = ctx.enter_context(nc.sbuf_tensor("y_sb", [P, 4096], FP16))
        ps = ctx.enter_context(nc.psum_tensor("ps", [P, 4096], FP32))
        # A DMA's completion semaphore only fires after the write RECEIPT of
        # every byte it moved -- measured 2.5-3.9us behind the data actually
        # landing while the stream is saturated. So each chunk is followed by
        # a tiny "canary" DMA on the same FIFO ring: each SDMA engine drains
        # the canary's descriptor after the chunk's descriptors, and its
        # writes are ordered behind the chunk's on the same SBUF port, so the
        # canary's (cheap, 64B/engine) completion implies the chunk landed.
        k_sems = [
            ctx.enter_context(nc.semaphore(f"k_sem{i}")) for i in range(NCHUNK)
        ]
        pe_sem = ctx.enter_context(nc.semaphore("pe_sem"))
        cpA = ctx.enter_context(nc.semaphore("cpA"))  # DVE copy halves
        cpB = ctx.enter_context(nc.semaphore("cpB"))  # ACT copy halves
        y_sem = ctx.enter_context(nc.semaphore("y_sem"))
        d_sem = ctx.enter_context(nc.semaphore("d_sem"))  # unwaited (DGE req.)
        block = ctx.enter_context(nc.Block())

        def y_out_wave(eng, groups, half):
            lo = half * 2048
            for a in groups:
                eng.dma_start(
                    y[2 * a : 2 * a + 2, lo : lo + 2048],
                    y_sb[Y_BASE[a] : Y_BASE[a] + 2, lo : lo + 2048],
                ).then_inc(y_sem, 16)

        @block.sync
        def _(sync):
            for c in range(NCHUNK):
                sync.dma_start(
                    x_sb[:, c * CCOL : (c + 1) * CCOL],
                    xp[:, c * CCOL : (c + 1) * CCOL],
                ).then_inc(d_sem, 16)
                sync.dma_start(can_sb[:], w[:]).then_inc(k_sems[c], 16)
            sync.wait_ge(cpA, 2)
            sync.wait_ge(cpB, 2)
            y_out_wave(sync, (0, 1), 0)
            sync.wait_ge(cpA, 4)
            sync.wait_ge(cpB, 4)
            y_out_wave(sync, (0, 1), 1)
            sync.wait_ge(y_sem, 128)

        @block.tensor
        def _(tensor):
            # HAM warmup: cold matmuls on garbage SBUF so the PE clock-gate
            # opens before the real work. Regions are overwritten by the real
            # matmuls (start=True resets PSUM).
            for k in range(N_WARM):
                src = (N_MM - N_WARM + k) * MM_N
                tensor.matmul(
                    ps[0:M_PAD, k * MM_N : (k + 1) * MM_N],
                    w_sb[:],
                    x_sb[:, src : src + MM_N],
                    start=True,
                    stop=True,
                )
            for c in range(NCHUNK):
                for j in range(MM_PER_CHUNK):
                    i = c * MM_PER_CHUNK + j
                    a, b = i % 4, i // 4
                    mm = tensor.matmul(
                        ps[32 * a : 32 * a + M_PAD, b * MM_N : (b + 1) * MM_N],
                        w_sb[:],
                        x_sb[:, i * MM_N : (i + 1) * MM_N],
                        start=True,
                        stop=True,
                        tile_position=(0, 32 * a),
                    )
                    if j == 0:
                        mm._wait_ge(k_sems[c], 32 if c == 0 else 16)
                    if j == MM_PER_CHUNK - 1:
                        mm.then_inc(pe_sem, 1)

        @block.vector
        def _(vector):
            # Left 512-column half of each chunk's PSUM region, f32 -> fp16.
            for c in range(NCHUNK):
                lo = c * 2 * MM_N
                vector.tensor_copy(
                    y_sb[:, lo : lo + MM_N], ps[:, lo : lo + MM_N]
                )._wait_ge(pe_sem, c + 1).then_inc(cpA, 1)

        @block.scalar
        def _(scalar):
            scalar.dma_start(w_sb[:], w[:]).then_inc(k_sems[0], 16)
            # Right 512-column half of each chunk's PSUM region.
            for c in range(NCHUNK):
                lo = c * 2 * MM_N + MM_N
                scalar.copy(
                    y_sb[:, lo : lo + MM_N], ps[:, lo : lo + MM_N]
                )._wait_ge(pe_sem, c + 1).then_inc(cpB, 1)
                if c == 1:
                    scalar.wait_ge(cpA, 2)
                    y_out_wave(scalar, (2, 3), 0)
            scalar.wait_ge(cpA, 4)
            y_out_wave(scalar, (2, 3), 1)

    return nc


def kernel(**inputs: np.ndarray) -> np.ndarray:
    x = np.asarray(inputs["x"], dtype=np.float32)
    m = _collapse_weights(
        np.asarray(inputs["W_enc"], dtype=np.float32),
        np.asarray(inputs["W_temp"], dtype=np.float32),
        np.asarray(inputs["W_dec"], dtype=np.float32),
        np.asarray(inputs["W_out"], dtype=np.float32),
    )
    m16 = m.astype(np.float16)
    w_np = np.zeros((P, M_PAD), dtype=np.float16)
    for col0 in (0, 4):
        w_np[0:H, col0] = m16
        w_np[H : 2 * H, col0 + 1] = m16

    # Pack x per core: [core, k = parity*64 + h, n] in fp16.
    x16 = x.astype(np.float16)
    xp_all = np.ascontiguousarray(
        x16.reshape(N_CORES, NCOL, 2, H).transpose(0, 2, 3, 1)
    ).reshape(N_CORES, P, NCOL)

    nc = _build_bass()
    in_maps = [{"xp": xp_all[i], "w": w_np} for i in range(N_CORES)]
    res = run_bass_kernel_spmd(
        nc, in_maps, core_ids=list(range(N_CORES)), **RUN_KWARGS
    )

    # Un-permute: y_dev[2a+m, 512b+j] = y[1024*(4b+a) + 2j + m].
    shard_b = B // N_CORES
    outs = []
    for r in res.results:
        yd = r["y"].astype(np.float32).reshape(4, 2, NCHUNK * 2, MM_N)
        y_core = np.ascontiguousarray(yd.transpose(2, 0, 3, 1)).reshape(ROWS)
        outs.append(y_core.reshape(shard_b, S, 1))
    return np.concatenate(outs, axis=0).astype(np.float32)


# revision 12
# speedup vs baseline: 1.2429x; 1.2429x over previous
"""Trainium2 kernel for nn_DeepLinearTimeSeries.

The reference network is a 400-layer *linear* residual MLP: every step is
x <- x @ (W_i^T) [+ 0.1 * carry], with no nonlinearities anywhere. The whole
stack therefore collapses algebraically to a single matrix:

    out = x @ M_total,   M_total = T_enc @ T_temp @ T_dec @ W_out^T  (64 x 1)

We fold the 400 64x64 factors on the host (trivial FLOPs), then run the
remaining memory-bound pass y = x @ m on 8 NeuronCores, data-parallel over
the batch dim (sharding_hint).

Device kernel v3 (PE matvec, fp16 stream):
  * x is packed on the host to fp16 [128, 16384] per core: column n holds
    rows (2n, 2n+1) interleaved over h -- k = parity*64 + h. This halves the
    HBM stream (4 MiB/core) and puts the contraction dim on SBUF partitions
    so the TensorEngine does the multiply+reduce:
        y[2n+p] = sum_k lhsT[k, p] * xpack[k, n]
  * lhsT [128, 32] holds the (even, odd) m-pair in columns (0,1) AND a
    duplicate in columns (4,5). 32 matmuls of N=512 go to PSUM regions
    [32a : 32a+32, bank b] with column-group a = i%4, bank b = i//4, so
    every PSUM bank is written across all 128 partitions and the 4
    column-group tiles execute CONCURRENTLY in the PE array (~4ns stagger).
    Useful rows per group alternate between the (0,1) and (4,5) pair so the
    4 output slices {0,1},{36,37},{64,65},{100,101} map to 4 *distinct*
    SDMA engines (pairs p and p+32 share an engine).
  * PSUM evacuation: per chunk, DVE copies the left 512 columns and ACT the
    right 512, casting f32 -> fp16 (halves the SBUF write traffic that
    competes with the input DMA stream for the 435 GB/s fabric).
  * y leaves as fp16 in 2 waves of 4 DMAs (wave 0 after chunks 0-1 are
    evacuated, hidden under the stream; wave 1 is the only tail). Host
    un-permutes and upcasts.
  * 6 warmup matmuls on garbage data at kernel start open the PE HAM clock
    gate before the real matmuls; the column-group concurrency gives PE a
    4x margin over the stream even when cold.
"""

import contextlib

import numpy as np

import concourse.bass as bass
import concourse.mybir as mybir
from concourse.bass_utils import run_bass_kernel_spmd

# Problem constants (hardcoded per harness contract).
B, S, H = 128, 2048, 64
N_CORES = 8
RW = np.float32(0.1)
ROWS = B * S // N_CORES          # 32768 rows per core
P = 128                          # SBUF partitions = 2 parities x 64 h
NCOL = ROWS // 2                 # 16384 packed moving columns per core
NCHUNK = 4
MM_N = 512                       # moving free dim per matmul (1 PSUM bank)
N_MM = 32
# Tapered DMA chunks (in matmuls): big while the pipe fills, small at the
# tail so the last chunk's land->compute->evacuate->output path is short.
CHUNK_MMS = [10, 10, 8, 4]
CHUNK_LO = [0, 10, 20, 28]
M_PAD = 32                       # stationary cols: pairs at (0,1),(4,5) + 0s
N_WARM = 6                       # HAM warmup matmuls
YCOL = NCOL // NCHUNK            # 4096 y columns per output row-pair
# Output slice base partitions: column-group a uses pair (0,1) for even a
# and (4,5) for odd a, landing the 4 slices on 4 distinct SDMA engines.
Y_BASE = [0, 36, 64, 100]
FP16 = mybir.dt.float16
FP32 = mybir.dt.float32

# Extra kwargs for run_bass_kernel_spmd (test harness sets these for tracing).
RUN_KWARGS: dict = {}


def _collapse_weights(W_enc, W_temp, W_dec, W_out):
    """Fold the full linear stack into a single [H] f32 vector."""
    eye = np.eye(H, dtype=np.float32)

    def block_mat(Ws):
        # x1 = x0 W0^T ; x2 = x1 W1^T + 0.1 x0 ; then x <- x (Wi^T + 0.1 I)
        T = Ws[0].T @ Ws[1].T + RW * eye
        for Wi in Ws[2:]:
            T = T @ (Wi.T + RW * eye)
        return T

    M = block_mat(W_enc) @ block_mat(W_temp)
    for Wd in W_dec:
        M = M @ (Wd.T + RW * eye)
    return (M @ W_out.T).astype(np.float32).reshape(H)  # [H]


def _build_bass():
    nc = bass.Bass()
    xp = nc.dram_tensor("xp", [P, NCOL], FP16, kind="ExternalInput")
    w = nc.dram_tensor("w", [P, M_PAD], FP16, kind="ExternalInput")
    y = nc.dram_tensor("y", [8, YCOL], FP16, kind="ExternalOutput")

    with contextlib.ExitStack() as ctx:
        w_sb = ctx.enter_context(nc.sbuf_tensor("w_sb", [P, M_PAD], FP16))
        can_sb = ctx.enter_context(nc.sbuf_tensor("can_sb", [1, 4096], FP16))
        x_sb = ctx.enter_context(nc.sbuf_tensor("x_sb", [P, NCOL], FP16))
        y_sb = ctx.enter_context(nc.sbuf_tensor("y_sb", [P, 4096], FP16))
        ps = ctx.enter_context(nc.psum_tensor("ps", [P, 4096], FP32))
        # A DMA's completion semaphore only fires after the write RECEIPT of
        # every byte it moved -- measured 2.5-3.9us behind the data actually
        # landing while the stream is saturated. So each chunk is followed by
        # a tiny "canary" DMA on the same FIFO ring: each SDMA engine drains
        # the canary's descriptor after the chunk's descriptors, and its
        # writes are ordered behind the chunk's on the same SBUF port, so the
        # canary's (cheap, 64B/engine) completion implies the chunk landed.
        k_sems = [
            ctx.enter_context(nc.semaphore(f"k_sem{i}")) for i in range(NCHUNK)
        ]
        pe_sem = ctx.enter_context(nc.semaphore("pe_sem"))
        cpA = ctx.enter_context(nc.semaphore("cpA"))  # DVE copy halves
        cpB = ctx.enter_context(nc.semaphore("cpB"))  # ACT copy halves
        y_sem = ctx.enter_context(nc.semaphore("y_sem"))
        d_sem = ctx.enter_context(nc.semaphore("d_sem"))  # unwaited (DGE req.)
        block = ctx.enter_context(nc.Block(no_gpsimd_drain=True))

        def y_out_wave(eng, groups, half):
            lo = half * 2048
            for a in groups:
                eng.dma_start(
                    y[2 * a : 2 * a + 2, lo : lo + 2048],
                    y_sb[Y_BASE[a] : Y_BASE[a] + 2, lo : lo + 2048],
                ).then_inc(y_sem, 16)

        @block.sync
        def _(sync):
            for c in range(NCHUNK):
                lo, hi = CHUNK_LO[c] * MM_N, (CHUNK_LO[c] + CHUNK_MMS[c]) * MM_N
                sync.dma_start(x_sb[:, lo:hi], xp[:, lo:hi]).then_inc(d_sem, 16)
                # 16-descriptor canary: [1, N] sprays one 512B descriptor to
                # each of the 16 SDMA queues (cheap to generate, 16 sem incs).
                sync.dma_start(can_sb[:], xp[0:1, 0:4096]).then_inc(
                    k_sems[c], 16
                )
            sync.wait_ge(cpA, 2)
            sync.wait_ge(cpB, 2)
            y_out_wave(sync, (0, 1), 0)
            sync.wait_ge(cpA, 4)
            sync.wait_ge(cpB, 4)
            y_out_wave(sync, (0, 1), 1)
            sync.wait_ge(y_sem, 128)

        @block.tensor
        def _(tensor):
            # HAM warmup: cold matmuls on garbage SBUF so the PE clock-gate
            # opens before the real work. Regions are overwritten by the real
            # matmuls (start=True resets PSUM).
            for k in range(N_WARM):
                src = (N_MM - N_WARM + k) * MM_N
                tensor.matmul(
                    ps[0:M_PAD, k * MM_N : (k + 1) * MM_N],
                    w_sb[:],
                    x_sb[:, src : src + MM_N],
                    start=True,
                    stop=True,
                )
            for c in range(NCHUNK):
                for j in range(CHUNK_MMS[c]):
                    i = CHUNK_LO[c] + j
                    a, b = i % 4, i // 4
                    mm = tensor.matmul(
                        ps[32 * a : 32 * a + M_PAD, b * MM_N : (b + 1) * MM_N],
                        w_sb[:],
                        x_sb[:, i * MM_N : (i + 1) * MM_N],
                        start=True,
                        stop=True,
                        tile_position=(0, 32 * a),
                    )
                    if j == 0:
                        mm._wait_ge(k_sems[c], 32 if c == 0 else 16)
                    if i % 4 == 3:
                        mm.then_inc(pe_sem, 1)  # bank i//4 complete

        @block.vector
        def _(vector):
            # Left 512-column half of each chunk's PSUM region, f32 -> fp16.
            for k in range(4):
                lo = k * 2 * MM_N
                vector.tensor_copy(
                    y_sb[:, lo : lo + MM_N], ps[:, lo : lo + MM_N]
                )._wait_ge(pe_sem, 2 * k + 1).then_inc(cpA, 1)

        @block.scalar
        def _(scalar):
            scalar.dma_start(w_sb[:], w[:]).then_inc(k_sems[0], 16)
            # Right 512-column half of each chunk's PSUM region.
            for k in range(4):
                lo = k * 2 * MM_N + MM_N
                scalar.copy(
                    y_sb[:, lo : lo + MM_N], ps[:, lo : lo + MM_N]
                )._wait_ge(pe_sem, 2 * k + 2).then_inc(cpB, 1)
                if k == 1:
                    scalar.wait_ge(cpA, 2)
                    y_out_wave(scalar, (2, 3), 0)
            scalar.wait_ge(cpA, 4)
            y_out_wave(scalar, (2, 3), 1)

    return nc


def kernel(**inputs: np.ndarray) -> np.ndarray:
    x = np.asarray(inputs["x"], dtype=np.float32)
    m = _collapse_weights(
        np.asarray(inputs["W_enc"], dtype=np.float32),
        np.asarray(inputs["W_temp"], dtype=np.float32),
        np.asarray(inputs["W_dec"], dtype=np.float32),
        np.asarray(inputs["W_out"], dtype=np.float32),
    )
    m16 = m.astype(np.float16)
    w_np = np.zeros((P, M_PAD), dtype=np.float16)
    for col0 in (0, 4):
        w_np[0:H, col0] = m16
        w_np[H : 2 * H, col0 + 1] = m16

    # Pack x per core: [core, k = parity*64 + h, n] in fp16.
    x16 = x.astype(np.float16)
    xp_all = np.ascontiguousarray(
        x16.reshape(N_CORES, NCOL, 2, H).transpose(0, 2, 3, 1)
    ).reshape(N_CORES, P, NCOL)

    nc = _build_bass()
    in_maps = [{"xp": xp_all[i], "w": w_np} for i in range(N_CORES)]
    res = run_bass_kernel_spmd(
        nc, in_maps, core_ids=list(range(N_CORES)), **RUN_KWARGS
    )

    # Un-permute: y_dev[2a+m, 512b+j] = y[1024*(4b+a) + 2j + m].
    shard_b = B // N_CORES
    outs = []
    for r in res.results:
        yd = r["y"].astype(np.float32).reshape(4, 2, NCHUNK * 2, MM_N)
        y_core = np.ascontiguousarray(yd.transpose(2, 0, 3, 1)).reshape(ROWS)
        outs.append(y_core.reshape(shard_b, S, 1))
    return np.concatenate(outs, axis=0).astype(np.float32)


# revision 15
# speedup vs baseline: 1.2941x; 1.0412x over previous
"""Trainium2 kernel for nn_DeepLinearTimeSeries.

The reference network is a 400-layer *linear* residual MLP: every step is
x <- x @ (W_i^T) [+ 0.1 * carry], with no nonlinearities anywhere. The whole
stack therefore collapses algebraically to a single matrix:

    out = x @ M_total,   M_total = T_enc @ T_temp @ T_dec @ W_out^T  (64 x 1)

We fold the 400 64x64 factors on the host (trivial FLOPs), then run the
remaining memory-bound pass y = x @ m on 8 NeuronCores, data-parallel over
the batch dim (sharding_hint).

Device kernel v3 (PE matvec, fp16 stream):
  * x is packed on the host to fp16 [128, 16384] per core: column n holds
    rows (2n, 2n+1) interleaved over h -- k = parity*64 + h. This halves the
    HBM stream (4 MiB/core) and puts the contraction dim on SBUF partitions
    so the TensorEngine does the multiply+reduce:
        y[2n+p] = sum_k lhsT[k, p] * xpack[k, n]
  * lhsT [128, 32] holds the (even, odd) m-pair in columns (0,1) AND a
    duplicate in columns (4,5). 32 matmuls of N=512 go to PSUM regions
    [32a : 32a+32, bank b] with column-group a = i%4, bank b = i//4, so
    every PSUM bank is written across all 128 partitions and the 4
    column-group tiles execute CONCURRENTLY in the PE array (~4ns stagger).
    Useful rows per group alternate between the (0,1) and (4,5) pair so the
    4 output slices {0,1},{36,37},{64,65},{100,101} map to 4 *distinct*
    SDMA engines (pairs p and p+32 share an engine).
  * PSUM evacuation: per chunk, DVE copies the left 512 columns and ACT the
    right 512, casting f32 -> fp16 (halves the SBUF write traffic that
    competes with the input DMA stream for the 435 GB/s fabric).
  * y leaves as fp16 in 2 waves of 4 DMAs (wave 0 after chunks 0-1 are
    evacuated, hidden under the stream; wave 1 is the only tail). Host
    un-permutes and upcasts.
  * 6 warmup matmuls on garbage data at kernel start open the PE HAM clock
    gate before the real matmuls; the column-group concurrency gives PE a
    4x margin over the stream even when cold.
"""

import contextlib

import numpy as np

import concourse.bass as bass
from concourse.ap import AP
import concourse.mybir as mybir
from concourse.bass_utils import run_bass_kernel_spmd

# Problem constants (hardcoded per harness contract).
B, S, H = 128, 2048, 64
N_CORES = 8
RW = np.float32(0.1)
ROWS = B * S // N_CORES          # 32768 rows per core
P = 128                          # SBUF partitions = 2 parities x 64 h
NCOL = ROWS // 2                 # 16384 packed moving columns per core
NCHUNK = 4
MM_N = 512                       # moving free dim per matmul (1 PSUM bank)
N_MM = 32
# Tapered DMA chunks (in matmuls): big while the pipe fills, small at the
# tail so the last chunk's land->compute->evacuate->output path is short.
CHUNK_MMS = [10, 10, 8, 4]
CHUNK_LO = [0, 10, 20, 28]
M_PAD = 32                       # stationary cols: pairs at (0,1),(4,5) + 0s
N_WARM = 6                       # HAM warmup matmuls
YCOL = NCOL // NCHUNK            # 4096 y columns per output row-pair
# Column-group a carries its m-pair in lhsT columns (4a, 4a+1), so the
# useful outputs sit at partitions 36a (even rows) and 36a+1 (odd rows) --
# a single stride-36 partition dim per output DMA, spread over 4 SDMA
# engines.
FP16 = mybir.dt.float16
FP32 = mybir.dt.float32

# Extra kwargs for run_bass_kernel_spmd (test harness sets these for tracing).
RUN_KWARGS: dict = {}


def _collapse_weights(W_enc, W_temp, W_dec, W_out):
    """Fold the full linear stack into a single [H] f32 vector."""
    eye = np.eye(H, dtype=np.float32)

    def block_mat(Ws):
        # x1 = x0 W0^T ; x2 = x1 W1^T + 0.1 x0 ; then x <- x (Wi^T + 0.1 I)
        T = Ws[0].T @ Ws[1].T + RW * eye
        for Wi in Ws[2:]:
            T = T @ (Wi.T + RW * eye)
        return T

    M = block_mat(W_enc) @ block_mat(W_temp)
    for Wd in W_dec:
        M = M @ (Wd.T + RW * eye)
    return (M @ W_out.T).astype(np.float32).reshape(H)  # [H]


def _build_bass():
    nc = bass.Bass()
    xp = nc.dram_tensor("xp", [P, NCOL], FP16, kind="ExternalInput")
    w = nc.dram_tensor("w", [P, M_PAD], FP16, kind="ExternalInput")
    y = nc.dram_tensor("y", [8, YCOL], FP16, kind="ExternalOutput")

    with contextlib.ExitStack() as ctx:
        w_sb = ctx.enter_context(nc.sbuf_tensor("w_sb", [P, M_PAD], FP16))
        can_sb = ctx.enter_context(nc.sbuf_tensor("can_sb", [1, 4096], FP16))
        x_sb = ctx.enter_context(nc.sbuf_tensor("x_sb", [P, NCOL], FP16))
        y_sb = ctx.enter_context(nc.sbuf_tensor("y_sb", [P, 4096], FP16))
        ps = ctx.enter_context(nc.psum_tensor("ps", [P, 4096], FP32))
        # A DMA's completion semaphore only fires after the write RECEIPT of
        # every byte it moved -- measured 2.5-3.9us behind the data actually
        # landing while the stream is saturated. So each chunk is followed by
        # a tiny "canary" DMA on the same FIFO ring: each SDMA engine drains
        # the canary's descriptor after the chunk's descriptors, and its
        # writes are ordered behind the chunk's on the same SBUF port, so the
        # canary's (cheap, 64B/engine) completion implies the chunk landed.
        k_sems = [
            ctx.enter_context(nc.semaphore(f"k_sem{i}")) for i in range(NCHUNK)
        ]
        pe_sem = ctx.enter_context(nc.semaphore("pe_sem"))
        cpA = ctx.enter_context(nc.semaphore("cpA"))  # DVE copy halves
        cpB = ctx.enter_context(nc.semaphore("cpB"))  # ACT copy halves
        y_sem = ctx.enter_context(nc.semaphore("y_sem"))
        d_sem = ctx.enter_context(nc.semaphore("d_sem"))  # unwaited (DGE req.)
        block = ctx.enter_context(nc.Block(no_gpsimd_drain=True))

        ysb_flat = y_sb[:]
        y_flat = y[:]

        def y_out(eng, parity, half):
            # One DMA per (parity, half): SBUF partitions {36a + parity} for
            # a in 0..3 (partition dim stride 36) -> y rows {2a + parity}.
            # For DMA, an SBUF AP's dim0 is THE partition dim (stride in
            # free-size units); remaining dims address within the partition.
            lo = half * 2048
            src_ap = AP(
                tensor=ysb_flat.tensor,
                offset=parity * 4096 + lo,
                ap=[[36 * 4096, 4], [1, 2048]],
            )
            dst_ap = AP(
                tensor=y_flat.tensor,
                offset=parity * 4096 + lo,
                ap=[[2 * 4096, 4], [1, 2048]],
            )
            eng.dma_start(dst_ap, src_ap).then_inc(y_sem, 16)

        @block.sync
        def _(sync):
            for c in range(NCHUNK):
                lo, hi = CHUNK_LO[c] * MM_N, (CHUNK_LO[c] + CHUNK_MMS[c]) * MM_N
                sync.dma_start(x_sb[:, lo:hi], xp[:, lo:hi]).then_inc(d_sem, 16)
                # 16-descriptor canary: [1, N] sprays one 512B descriptor to
                # each of the 16 SDMA queues (cheap to generate, 16 sem incs).
                sync.dma_start(can_sb[:], xp[0:1, 0:512]).then_inc(
                    k_sems[c], 16
                )
            sync.wait_ge(cpA, 2)
            sync.wait_ge(cpB, 2)
            y_out(sync, 0, 0)
            y_out(sync, 1, 0)
            sync.wait_ge(cpA, 4)
            sync.wait_ge(cpB, 4)
            y_out(sync, 0, 1)
            sync.wait_ge(y_sem, 64)

        @block.tensor
        def _(tensor):
            # HAM warmup: cold matmuls on garbage SBUF so the PE clock-gate
            # opens before the real work. Regions are overwritten by the real
            # matmuls (start=True resets PSUM).
            for k in range(N_WARM):
                src = (N_MM - N_WARM + k) * MM_N
                tensor.matmul(
                    ps[0:M_PAD, k * MM_N : (k + 1) * MM_N],
                    w_sb[:],
                    x_sb[:, src : src + MM_N],
                    start=True,
                    stop=True,
                )
            for c in range(NCHUNK):
                for j in range(CHUNK_MMS[c]):
                    i = CHUNK_LO[c] + j
                    a, b = i % 4, i // 4
                    mm = tensor.matmul(
                        ps[32 * a : 32 * a + M_PAD, b * MM_N : (b + 1) * MM_N],
                        w_sb[:],
                        x_sb[:, i * MM_N : (i + 1) * MM_N],
                        start=True,
                        stop=True,
                        tile_position=(0, 32 * a),
                    )
                    if j == 0:
                        mm._wait_ge(k_sems[c], 32 if c == 0 else 16)
                    if i % 4 == 3:
                        mm.then_inc(pe_sem, 1)  # bank i//4 complete

        @block.vector
        def _(vector):
            # Left 512-column half of each chunk's PSUM region, f32 -> fp16.
            for k in range(4):
                lo = k * 2 * MM_N
                vector.tensor_copy(
                    y_sb[:, lo : lo + MM_N], ps[:, lo : lo + MM_N]
                )._wait_ge(pe_sem, 2 * k + 1).then_inc(cpA, 1)

        @block.scalar
        def _(scalar):
            scalar.dma_start(w_sb[:], w[:]).then_inc(k_sems[0], 16)
            # Right 512-column half of each chunk's PSUM region.
            for k in range(4):
                lo = k * 2 * MM_N + MM_N
                scalar.copy(
                    y_sb[:, lo : lo + MM_N], ps[:, lo : lo + MM_N]
                )._wait_ge(pe_sem, 2 * k + 2).then_inc(cpB, 1)
            scalar.wait_ge(cpA, 4)
            y_out(scalar, 1, 1)

    return nc


def kernel(**inputs: np.ndarray) -> np.ndarray:
    x = np.asarray(inputs["x"], dtype=np.float32)
    m = _collapse_weights(
        np.asarray(inputs["W_enc"], dtype=np.float32),
        np.asarray(inputs["W_temp"], dtype=np.float32),
        np.asarray(inputs["W_dec"], dtype=np.float32),
        np.asarray(inputs["W_out"], dtype=np.float32),
    )
    m16 = m.astype(np.float16)
    w_np = np.zeros((P, M_PAD), dtype=np.float16)
    for a in range(4):
        w_np[0:H, 4 * a] = m16
        w_np[H : 2 * H, 4 * a + 1] = m16

    # Pack x per core: [core, k = parity*64 + h, n] in fp16.
    x16 = x.astype(np.float16)
    xp_all = np.ascontiguousarray(
        x16.reshape(N_CORES, NCOL, 2, H).transpose(0, 2, 3, 1)
    ).reshape(N_CORES, P, NCOL)

    nc = _build_bass()
    in_maps = [{"xp": xp_all[i], "w": w_np} for i in range(N_CORES)]
    res = run_bass_kernel_spmd(
        nc, in_maps, core_ids=list(range(N_CORES)), **RUN_KWARGS
    )

    # Un-permute: y_dev[2a+m, 512b+j] = y[1024*(4b+a) + 2j + m].
    shard_b = B // N_CORES
    outs = []
    for r in res.results:
        yd = r["y"].astype(np.float32).reshape(4, 2, NCHUNK * 2, MM_N)
        y_core = np.ascontiguousarray(yd.transpose(2, 0, 3, 1)).reshape(ROWS)
        outs.append(y_core.reshape(shard_b, S, 1))
    return np.concatenate(outs, axis=0).astype(np.float32)


# revision 17
# speedup vs baseline: 1.3499x; 1.0431x over previous
"""Trainium2 kernel for nn_DeepLinearTimeSeries.

The reference network is a 400-layer *linear* residual MLP: every step is
x <- x @ (W_i^T) [+ 0.1 * carry], with no nonlinearities anywhere. The whole
stack therefore collapses algebraically to a single matrix:

    out = x @ M_total,   M_total = T_enc @ T_temp @ T_dec @ W_out^T  (64 x 1)

We fold the 400 64x64 factors on the host (trivial FLOPs), then run the
remaining memory-bound pass y = x @ m on 8 NeuronCores, data-parallel over
the batch dim (sharding_hint).

Device kernel v3 (PE matvec, fp16 stream):
  * x is packed on the host to fp16 [128, 16384] per core: column n holds
    rows (2n, 2n+1) interleaved over h -- k = parity*64 + h. This halves the
    HBM stream (4 MiB/core) and puts the contraction dim on SBUF partitions
    so the TensorEngine does the multiply+reduce:
        y[2n+p] = sum_k lhsT[k, p] * xpack[k, n]
  * lhsT [128, 32] holds the (even, odd) m-pair in columns (0,1) AND a
    duplicate in columns (4,5). 32 matmuls of N=512 go to PSUM regions
    [32a : 32a+32, bank b] with column-group a = i%4, bank b = i//4, so
    every PSUM bank is written across all 128 partitions and the 4
    column-group tiles execute CONCURRENTLY in the PE array (~4ns stagger).
    Useful rows per group alternate between the (0,1) and (4,5) pair so the
    4 output slices {0,1},{36,37},{64,65},{100,101} map to 4 *distinct*
    SDMA engines (pairs p and p+32 share an engine).
  * PSUM evacuation: per chunk, DVE copies the left 512 columns and ACT the
    right 512, casting f32 -> fp16 (halves the SBUF write traffic that
    competes with the input DMA stream for the 435 GB/s fabric).
  * y leaves as fp16 in 2 waves of 4 DMAs (wave 0 after chunks 0-1 are
    evacuated, hidden under the stream; wave 1 is the only tail). Host
    un-permutes and upcasts.
  * 6 warmup matmuls on garbage data at kernel start open the PE HAM clock
    gate before the real matmuls; the column-group concurrency gives PE a
    4x margin over the stream even when cold.
"""

import contextlib

import numpy as np

import concourse.bass as bass
from concourse.ap import AP
import concourse.mybir as mybir
from concourse.bass_utils import run_bass_kernel_spmd

# Problem constants (hardcoded per harness contract).
B, S, H = 128, 2048, 64
N_CORES = 8
RW = np.float32(0.1)
ROWS = B * S // N_CORES          # 32768 rows per core
P = 128                          # SBUF partitions = 2 parities x 64 h
NCOL = ROWS // 2                 # 16384 packed moving columns per core
NCHUNK = 3
MM_N = 512                       # moving free dim per matmul (1 PSUM bank)
N_MM = 32
# Tapered DMA chunks (in matmuls): big while the pipe fills, small at the
# tail so the last chunk's land->compute->evacuate->output path is short.
# Few chunks: each dma_start costs ~0.8us of serialized descriptor-gen plus
# ~1.5-2.7us of descriptor-visibility lag before the SDMA engines see it.
CHUNK_MMS = [16, 12, 4]
CHUNK_LO = [0, 16, 28]
M_PAD = 32                       # stationary cols: pairs at (0,1),(4,5) + 0s
N_WARM = 6                       # HAM warmup matmuls
YCOL = N_MM * MM_N // 4          # 4096 y columns per output row-pair
# Column-group a carries its m-pair in lhsT columns (4a, 4a+1), so the
# useful outputs sit at partitions 36a (even rows) and 36a+1 (odd rows) --
# a single stride-36 partition dim per output DMA, spread over 4 SDMA
# engines.
FP16 = mybir.dt.float16
FP32 = mybir.dt.float32

# Extra kwargs for run_bass_kernel_spmd (test harness sets these for tracing).
RUN_KWARGS: dict = {}


def _collapse_weights(W_enc, W_temp, W_dec, W_out):
    """Fold the full linear stack into a single [H] f32 vector."""
    eye = np.eye(H, dtype=np.float32)

    def block_mat(Ws):
        # x1 = x0 W0^T ; x2 = x1 W1^T + 0.1 x0 ; then x <- x (Wi^T + 0.1 I)
        T = Ws[0].T @ Ws[1].T + RW * eye
        for Wi in Ws[2:]:
            T = T @ (Wi.T + RW * eye)
        return T

    M = block_mat(W_enc) @ block_mat(W_temp)
    for Wd in W_dec:
        M = M @ (Wd.T + RW * eye)
    return (M @ W_out.T).astype(np.float32).reshape(H)  # [H]


def _build_bass():
    nc = bass.Bass()
    xp = nc.dram_tensor("xp", [P, NCOL], FP16, kind="ExternalInput")
    w = nc.dram_tensor("w", [P, M_PAD], FP16, kind="ExternalInput")
    y = nc.dram_tensor("y", [8, YCOL], FP16, kind="ExternalOutput")

    with contextlib.ExitStack() as ctx:
        w_sb = ctx.enter_context(nc.sbuf_tensor("w_sb", [P, M_PAD], FP16))
        can_sb = ctx.enter_context(nc.sbuf_tensor("can_sb", [1, 4096], FP16))
        x_sb = ctx.enter_context(nc.sbuf_tensor("x_sb", [P, NCOL], FP16))
        y_sb = ctx.enter_context(nc.sbuf_tensor("y_sb", [P, 4096], FP16))
        ps = ctx.enter_context(nc.psum_tensor("ps", [P, 4096], FP32))
        # A DMA's completion semaphore only fires after the write RECEIPT of
        # every byte it moved -- measured 2.5-3.9us behind the data actually
        # landing while the stream is saturated. So each chunk is followed by
        # a tiny "canary" DMA on the same FIFO ring: each SDMA engine drains
        # the canary's descriptor after the chunk's descriptors, and its
        # writes are ordered behind the chunk's on the same SBUF port, so the
        # canary's (cheap, 64B/engine) completion implies the chunk landed.
        k_sems = [
            ctx.enter_context(nc.semaphore(f"k_sem{i}")) for i in range(NCHUNK)
        ]
        pe_sem = ctx.enter_context(nc.semaphore("pe_sem"))
        cpA = ctx.enter_context(nc.semaphore("cpA"))  # DVE copy halves
        cpB = ctx.enter_context(nc.semaphore("cpB"))  # ACT copy halves
        y_sem = ctx.enter_context(nc.semaphore("y_sem"))
        d_sem = ctx.enter_context(nc.semaphore("d_sem"))  # unwaited (DGE req.)
        block = ctx.enter_context(nc.Block(no_gpsimd_drain=True))

        ysb_flat = y_sb[:]
        y_flat = y[:]

        def y_out(eng, parity, half):
            # One DMA per (parity, half): SBUF partitions {36a + parity} for
            # a in 0..3 (partition dim stride 36) -> y rows {2a + parity}.
            # For DMA, an SBUF AP's dim0 is THE partition dim (stride in
            # free-size units); remaining dims address within the partition.
            lo = half * 2048
            src_ap = AP(
                tensor=ysb_flat.tensor,
                offset=parity * 4096 + lo,
                ap=[[36 * 4096, 4], [1, 2048]],
            )
            dst_ap = AP(
                tensor=y_flat.tensor,
                offset=parity * 4096 + lo,
                ap=[[2 * 4096, 4], [1, 2048]],
            )
            eng.dma_start(dst_ap, src_ap).then_inc(y_sem, 16)

        @block.sync
        def _(sync):
            for c in range(NCHUNK):
                lo, hi = CHUNK_LO[c] * MM_N, (CHUNK_LO[c] + CHUNK_MMS[c]) * MM_N
                sync.dma_start(x_sb[:, lo:hi], xp[:, lo:hi]).then_inc(d_sem, 16)
                # Canary: full-width [128, 32] so each SDMA engine writes
                # its own partitions (8x 32B descriptors); its sem releases
                # ~0.7us after the data vs 2-3us for narrow/cross-port shapes.
                sync.dma_start(can_sb[:], xp[:, 0:M_PAD]).then_inc(
                    k_sems[c], 16
                )
            sync.wait_ge(cpA, 2)
            sync.wait_ge(cpB, 2)
            y_out(sync, 0, 0)
            y_out(sync, 1, 0)
            sync.wait_ge(cpA, 4)
            sync.wait_ge(cpB, 4)
            y_out(sync, 0, 1)
            sync.wait_ge(y_sem, 64)

        @block.tensor
        def _(tensor):
            # HAM warmup: cold matmuls on garbage SBUF so the PE clock-gate
            # opens before the real work. Regions are overwritten by the real
            # matmuls (start=True resets PSUM).
            for k in range(N_WARM):
                src = (N_MM - N_WARM + k) * MM_N
                tensor.matmul(
                    ps[0:M_PAD, k * MM_N : (k + 1) * MM_N],
                    w_sb[:],
                    x_sb[:, src : src + MM_N],
                    start=True,
                    stop=True,
                )
            for c in range(NCHUNK):
                for j in range(CHUNK_MMS[c]):
                    i = CHUNK_LO[c] + j
                    a, b = i % 4, i // 4
                    mm = tensor.matmul(
                        ps[32 * a : 32 * a + M_PAD, b * MM_N : (b + 1) * MM_N],
                        w_sb[:],
                        x_sb[:, i * MM_N : (i + 1) * MM_N],
                        start=True,
                        stop=True,
                        tile_position=(0, 32 * a),
                    )
                    if j == 0:
                        mm._wait_ge(k_sems[c], 32 if c == 0 else 16)
                    if i % 4 == 3:
                        mm.then_inc(pe_sem, 1)  # bank i//4 complete

        @block.vector
        def _(vector):
            # Left 512-column half of each chunk's PSUM region, f32 -> fp16.
            for k in range(4):
                lo = k * 2 * MM_N
                vector.tensor_copy(
                    y_sb[:, lo : lo + MM_N], ps[:, lo : lo + MM_N]
                )._wait_ge(pe_sem, 2 * k + 1).then_inc(cpA, 1)

        @block.scalar
        def _(scalar):
            scalar.dma_start(w_sb[:], w[:]).then_inc(k_sems[0], 16)
            # Right 512-column half of each chunk's PSUM region.
            for k in range(4):
                lo = k * 2 * MM_N + MM_N
                scalar.copy(
                    y_sb[:, lo : lo + MM_N], ps[:, lo : lo + MM_N]
                )._wait_ge(pe_sem, 2 * k + 2).then_inc(cpB, 1)
            scalar.wait_ge(cpA, 4)
            y_out(scalar, 1, 1)

    return nc


def kernel(**inputs: np.ndarray) -> np.ndarray:
    x = np.asarray(inputs["x"], dtype=np.float32)
    m = _collapse_weights(
        np.asarray(inputs["W_enc"], dtype=np.float32),
        np.asarray(inputs["W_temp"], dtype=np.float32),
        np.asarray(inputs["W_dec"], dtype=np.float32),
        np.asarray(inputs["W_out"], dtype=np.float32),
    )
    m16 = m.astype(np.float16)
    w_np = np.zeros((P, M_PAD), dtype=np.float16)
    for a in range(4):
        w_np[0:H, 4 * a] = m16
        w_np[H : 2 * H, 4 * a + 1] = m16

    # Pack x per core: [core, k = parity*64 + h, n] in fp16.
    x16 = x.astype(np.float16)
    xp_all = np.ascontiguousarray(
        x16.reshape(N_CORES, NCOL, 2, H).transpose(0, 2, 3, 1)
    ).reshape(N_CORES, P, NCOL)

    nc = _build_bass()
    in_maps = [{"xp": xp_all[i], "w": w_np} for i in range(N_CORES)]
    res = run_bass_kernel_spmd(
        nc, in_maps, core_ids=list(range(N_CORES)), **RUN_KWARGS
    )

    # Un-permute: y_dev[2a+m, 512b+j] = y[1024*(4b+a) + 2j + m].
    shard_b = B // N_CORES
    outs = []
    for r in res.results:
        yd = r["y"].astype(np.float32).reshape(4, 2, 8, MM_N)
        y_core = np.ascontiguousarray(yd.transpose(2, 0, 3, 1)).reshape(ROWS)
        outs.append(y_core.reshape(shard_b, S, 1))
    return np.concatenate(outs, axis=0).astype(np.float32)
